# revision 53
# baseline (speedup 1.0000x reference)
"""Circulant matmul for TRN2: 4-level CRT with trinomial split, bf16 matmuls.

out[b, r] = sum_c x[b,c] w[(c-r) mod N] = (x (*) wt)[r], wt = roll(w[::-1],1)
(cyclic convolution along c) — no input flip or output reversal needed.

CRT tree on z^4096-1 (per 128-row block of x):
  cyc4096 -> cyc2048 + nega2048             (binomial, band scale 1/2 each)
  cyc2048 -> cyc1024 + nega1024[LEAF]       (1/2)
  cyc1024 -> cyc512[LEAF] + nega512[LEAF]   (1/2)
  nega2048 -> T1024+ , T1024- [LEAVES]      (z^1024 -+ sqrt2 z^512 + 1, 1/(2sqrt2))
Leaf matmuls: 56 x [K=128, M=128, N=512] bf16 = 28672 PE cycles/block
(vs 88/45056 for the 3-level f32r baseline). Reconstruction scales fold into
the ACT PSUM->SBUF evacuations; sqrt2 scale passes run on ACT. Ring bands are
(piecewise) shear tensors S[q - k]; the host precomputes all 7 leaf strips
from w into one [128, 9344] bf16 table (2.4 MB) — no on-chip band build.

Per-block engine budget: PE 11.95us (critical), DMA 11.65, DVE ~11
(bf16 2x tensor_tensor), Pool ~10.3 (plain tensor_tensor only — the
compiler rejects scalar_tensor_tensor/tensor_scalar on Pool), ACT ~7.
Evacs interleave with the matmul stream (PSUM banks recycle without
stalling PE); the last block runs T matmuls first and nega1024 last in
two half-PSUM tiles so the reconstruction tail streams out per half.
"""

import sys

sys.path.insert(0, "/opt/trn_rl_repo")

import numpy as np

N = 4096
B = 8192
N_CORES = 8
B_SHARD = B // N_CORES  # 1024
NB = B_SHARD // 128     # 8 row-tiles per core
R2 = float(np.sqrt(2.0))
SC_T = float(1.0 / (4.0 * np.sqrt(2.0)))   # T1024 leaves: 1/2 * 1/(2sqrt2)

# strip layout in the packed [128, SBW] host table (name: col0, width, OFF)
STRIPS = {
    "sC5": (0, 896, -384),
    "sN5": (896, 896, -384),
    "sN10": (1792, 1920, -896),
    "sLp": (3712, 1408, -896),
    "sHp": (5120, 1408, -896),
    "sLm": (6528, 1408, -896),
    "sHm": (7936, 1408, -896),
}
SBW = 9344
SB_SPLIT = 3712  # first DMA: C5/N5/N10 strips; second: T strips

_STATE = {}


def _build():
    import concourse.bacc as bacc
    import concourse.mybir as mybir
    import concourse.tile as tile

    f32 = mybir.dt.float32
    bf16 = mybir.dt.bfloat16
    ADD = mybir.AluOpType.add
    SUB = mybir.AluOpType.subtract

    nc = bacc.Bacc("TRN2", target_bir_lowering=False, debug=False)
    xtr_d = nc.declare_dram_parameter("xtr", [N, B_SHARD], f32, isOutput=False)
    sb_d = nc.declare_dram_parameter("sb", [128, SBW], bf16, isOutput=False)
    out_d = nc.declare_dram_parameter("out", [B_SHARD, N], f32, isOutput=True)

    xtr_t = xtr_d[:].rearrange("(a p) b -> p a b", p=128)  # [128, 32, B_SHARD]

    with tile.TileContext(nc) as tc:
        with (
            tc.tile_pool(name="const", bufs=1) as constp,
            tc.tile_pool(name="xbig", bufs=3) as xbigp,
            tc.tile_pool(name="fold", bufs=2) as fp,
            tc.tile_pool(name="evac", bufs=3) as ep,
            tc.tile_pool(name="unf", bufs=2) as up,
            tc.tile_pool(name="outp", bufs=2) as op,
            tc.tile_pool(name="psum", bufs=1, space="PSUM") as pp,
        ):
            SB = constp.tile([128, SBW], bf16, name="SB")
            warm = constp.tile([128, 512], bf16, name="warm")
            strip = {k: SB[:, c0 : c0 + w] for k, (c0, w, _) in STRIPS.items()}

            def emit_dma_in(bt):
                b0 = 128 * bt
                xbig = xbigp.tile([128, 32, 128], f32, tag="xbig", name="xbig")
                nc.sync.dma_start(xbig[:], xtr_t[:, :, b0 : b0 + 128])
                return xbig

            def emit_folds_a(xbig, swap=False):
                """Early folds: xc (Pool), binomial chain (DVE) — these feed
                the first matmuls of the next block. swap=True runs xc on DVE
                instead (fill phase: xc gates the chain and Pool is 2x slower
                at the mixed L1 op)."""
                xc = fp.tile([128, 16, 128], bf16, tag="xc", name="xc")
                xn = fp.tile([128, 16, 128], bf16, tag="xn", name="xn")
                if swap:
                    nc.vector.tensor_tensor(
                        xc[:], xbig[:, 0:32:2, :], xbig[:, 1:32:2, :], ADD
                    )
                    nc.gpsimd.tensor_tensor(
                        xn[:], xbig[:, 0:32:2, :], xbig[:, 1:32:2, :], SUB
                    )
                else:
                    nc.gpsimd.tensor_tensor(
                        xc[:], xbig[:, 0:32:2, :], xbig[:, 1:32:2, :], ADD
                    )
                    nc.vector.tensor_tensor(
                        xn[:], xbig[:, 0:32:2, :], xbig[:, 1:32:2, :], SUB
                    )
                xcc = fp.tile([128, 8, 128], bf16, tag="xcc", name="xcc")
                nc.vector.tensor_tensor(xcc[:], xc[:, 0:8, :], xc[:, 8:16, :], ADD)
                xccc = fp.tile([128, 4, 128], bf16, tag="xccc", name="xccc")
                nc.vector.tensor_tensor(
                    xccc[:], xcc[:, 0:4, :], xcc[:, 4:8, :], ADD
                )
                xccn = fp.tile([128, 4, 128], bf16, tag="xccn", name="xccn")
                nc.vector.tensor_tensor(
                    xccn[:], xcc[:, 0:4, :], xcc[:, 4:8, :], SUB
                )
                xcn = fp.tile([128, 8, 128], bf16, tag="xcn", name="xcn")
                nc.vector.tensor_tensor(xcn[:], xc[:, 0:8, :], xc[:, 8:16, :], SUB)
                return dict(xn=xn, xcn=xcn, xccc=xccc, xccn=xccn)

            def emit_folds_b(fa):
                """Late folds: trinomial xTp/xTm (needed only by the T
                matmuls, ~5us into the block). sqrt2 scales on ACT."""
                xn = fa["xn"]
                xA = fp.tile([128, 4, 128], bf16, tag="xA", name="xA")
                nc.vector.tensor_tensor(xA[:], xn[:, 0:4, :], xn[:, 8:12, :], SUB)
                xB = fp.tile([128, 4, 128], bf16, tag="xB", name="xB")
                nc.vector.tensor_tensor(xB[:], xn[:, 4:8, :], xn[:, 12:16, :], ADD)
                s10 = fp.tile([128, 4, 128], bf16, tag="s10", name="s10")
                nc.scalar.mul(s10[:], xn[:, 8:12, :], R2)
                s11 = fp.tile([128, 4, 128], bf16, tag="s11", name="s11")
                nc.scalar.mul(s11[:], xn[:, 12:16, :], R2)
                xTp = fp.tile([128, 8, 128], bf16, tag="xTp", name="xTp")
                nc.vector.tensor_tensor(xTp[:, 0:4, :], xA[:], s11[:], ADD)
                nc.vector.tensor_tensor(xTp[:, 4:8, :], xB[:], s10[:], SUB)
                xTm = fp.tile([128, 8, 128], bf16, tag="xTm", name="xTm")
                nc.vector.tensor_tensor(xTm[:, 0:4, :], xA[:], s11[:], SUB)
                nc.vector.tensor_tensor(xTm[:, 4:8, :], xB[:], s10[:], ADD)
                fa["xTp"] = xTp
                fa["xTm"] = xTm
                return fa

            def emit_folds(xbig, first=False):
                # Block 0 only: fold each interleaved DMA half as it lands;
                # xc on DVE (the cyc chain gates the first matmuls).
                xc = fp.tile([128, 16, 128], bf16, tag="xc", name="xc")
                xn = fp.tile([128, 16, 128], bf16, tag="xn", name="xn")
                for h in range(2):
                    nc.vector.tensor_tensor(
                        xc[:, 8 * h : 8 * h + 8, :],
                        xbig[:, 16 * h : 16 * h + 16 : 2, :],
                        xbig[:, 16 * h + 1 : 16 * h + 16 : 2, :], ADD
                    )
                    nc.gpsimd.tensor_tensor(
                        xn[:, 8 * h : 8 * h + 8, :],
                        xbig[:, 16 * h : 16 * h + 16 : 2, :],
                        xbig[:, 16 * h + 1 : 16 * h + 16 : 2, :], SUB
                    )
                # L2/L3 binomial on xc (DVE); C5/N5 operands first, xcn
                # (needed only by the mid-block N10 matmuls) last
                xcc = fp.tile([128, 8, 128], bf16, tag="xcc", name="xcc")
                nc.vector.tensor_tensor(xcc[:], xc[:, 0:8, :], xc[:, 8:16, :], ADD)
                xccc = fp.tile([128, 4, 128], bf16, tag="xccc", name="xccc")
                nc.vector.tensor_tensor(
                    xccc[:], xcc[:, 0:4, :], xcc[:, 4:8, :], ADD
                )
                xccn = fp.tile([128, 4, 128], bf16, tag="xccn", name="xccn")
                nc.vector.tensor_tensor(
                    xccn[:], xcc[:, 0:4, :], xcc[:, 4:8, :], SUB
                )
                xcn = fp.tile([128, 8, 128], bf16, tag="xcn", name="xcn")
                nc.vector.tensor_tensor(xcn[:], xc[:, 0:8, :], xc[:, 8:16, :], SUB)
                # trinomial fold of xn -> xTp, xTm (sqrt2 scales on ACT)
                xA = fp.tile([128, 4, 128], bf16, tag="xA", name="xA")
                nc.vector.tensor_tensor(xA[:], xn[:, 0:4, :], xn[:, 8:12, :], SUB)
                xB = fp.tile([128, 4, 128], bf16, tag="xB", name="xB")
                nc.vector.tensor_tensor(xB[:], xn[:, 4:8, :], xn[:, 12:16, :], ADD)
                s10 = fp.tile([128, 4, 128], bf16, tag="s10", name="s10")
                nc.scalar.mul(s10[:], xn[:, 8:12, :], R2)
                s11 = fp.tile([128, 4, 128], bf16, tag="s11", name="s11")
                nc.scalar.mul(s11[:], xn[:, 12:16, :], R2)
                xTp = fp.tile([128, 8, 128], bf16, tag="xTp", name="xTp")
                nc.vector.tensor_tensor(xTp[:, 0:4, :], xA[:], s11[:], ADD)
                nc.vector.tensor_tensor(xTp[:, 4:8, :], xB[:], s10[:], SUB)
                xTm = fp.tile([128, 8, 128], bf16, tag="xTm", name="xTm")
                nc.vector.tensor_tensor(xTm[:, 0:4, :], xA[:], s11[:], SUB)
                nc.vector.tensor_tensor(xTm[:, 4:8, :], xB[:], s10[:], ADD)
                return dict(xcn=xcn, xTp=xTp, xTm=xTm, xccc=xccc, xccn=xccn)

            def mm_ring(psum, xres, sname, nchunks):
                s = strip[sname]
                for a in range(nchunks):
                    v0 = 128 * (nchunks - 1) - 128 * a
                    nc.tensor.matmul(
                        psum[:], xres[:, a, :], s[:, v0 : v0 + 512],
                        start=(a == 0), stop=(a == nchunks - 1),
                    )

            def mms_cn5(f):
                pC5 = pp.tile([128, 512], f32, tag="pc5", name="pc5")
                mm_ring(pC5, f["xccc"], "sC5", 4)
                pN5 = pp.tile([128, 512], f32, tag="pn5", name="pn5")
                mm_ring(pN5, f["xccn"], "sN5", 4)
                rCN5 = ep.tile([128, 1024], bf16, tag="rcn5", name="rcn5")
                nc.scalar.mul(rCN5[:, 0:512], pC5[:], 0.125)
                nc.scalar.mul(rCN5[:, 512:1024], pN5[:], 0.125)
                return rCN5

            def mms_n10(f):
                """nega1024 in two half-PSUM tiles (tail streams per half)."""
                pN10a = pp.tile([128, 512], f32, tag="pn10a", name="pn10a")
                pN10b = pp.tile([128, 512], f32, tag="pn10b", name="pn10b")
                sN10 = strip["sN10"]
                rN10 = ep.tile([128, 1024], bf16, tag="rn10", name="rn10")
                for j, ps in ((0, pN10a), (1, pN10b)):
                    for a in range(8):
                        v0 = 896 - 128 * a + 512 * j
                        nc.tensor.matmul(
                            ps[:], f["xcn"][:, a, :], sN10[:, v0 : v0 + 512],
                            start=(a == 0), stop=(a == 7),
                        )
                    nc.scalar.mul(rN10[:, 512 * j : 512 * j + 512], ps[:], 0.25)
                return rN10

            def mms_t(f):
                rTp = ep.tile([128, 1024], bf16, tag="rtp", name="rtp")
                rTm = ep.tile([128, 1024], bf16, tag="rtm", name="rtm")
                for xres, rr, lo_name, hi_name in (
                    (f["xTp"], rTp, "sLp", "sHp"),
                    (f["xTm"], rTm, "sLm", "sHm"),
                ):
                    for j, sname in ((0, lo_name), (1, hi_name)):
                        ph = pp.tile(
                            [128, 512], f32, tag=f"pt{sname}", name=f"pt{sname}"
                        )
                        mm_ring(ph, xres, sname, 8)
                        nc.scalar.mul(
                            rr[:, 512 * j : 512 * j + 512], ph[:], SC_T
                        )
                return rTp, rTm

            def emit_rn_chain(rTp, rTm):
                """nega2048 reconstruction from T1024+/- (DVE + ACT scale)."""
                rn = up.tile([128, 2048], bf16, tag="rn", name="rn")
                tt01 = up.tile([128, 1024], bf16, tag="tt01", name="tt01")
                st01 = up.tile([128, 1024], bf16, tag="st01", name="st01")
                nc.vector.tensor_tensor(  # o3
                    rn[:, 1536:2048], rTp[:, 0:512], rTm[:, 0:512], SUB
                )
                nc.vector.tensor_tensor(  # t0
                    tt01[:, 0:512], rTp[:, 0:512], rTm[:, 0:512], ADD
                )
                nc.vector.tensor_tensor(  # o2
                    rn[:, 1024:1536], rTm[:, 512:1024], rTp[:, 512:1024], SUB
                )
                nc.vector.tensor_tensor(  # t1
                    tt01[:, 512:1024], rTp[:, 512:1024], rTm[:, 512:1024], ADD
                )
                nc.scalar.mul(st01[:], tt01[:], R2)
                nc.vector.tensor_tensor(  # o0
                    rn[:, 0:512], st01[:, 0:512], rn[:, 1024:1536], ADD
                )
                nc.vector.tensor_tensor(  # o1
                    rn[:, 512:1024], st01[:, 512:1024], rn[:, 1536:2048], SUB
                )
                return rn

            def emit_rc_part(rCN5, rN10):
                rc1 = up.tile([128, 1024], bf16, tag="rc1", name="rc1")
                nc.vector.tensor_tensor(
                    rc1[:, 0:512], rCN5[:, 0:512], rCN5[:, 512:1024], ADD
                )
                nc.vector.tensor_tensor(
                    rc1[:, 512:1024], rCN5[:, 0:512], rCN5[:, 512:1024], SUB
                )
                rc2 = up.tile([128, 2048], bf16, tag="rc2", name="rc2")
                nc.vector.tensor_tensor(rc2[:, 0:1024], rc1[:], rN10[:], ADD)
                nc.vector.tensor_tensor(rc2[:, 1024:2048], rc1[:], rN10[:], SUB)
                return rc2

            def emit_finals(bt, rc2, rn, halves=False):
                b0 = 128 * bt
                specs = ((ADD, 0, "D"), (ADD, 1024, "P"), (SUB, 0, "D"), (SUB, 1024, "P"))
                os_ = [
                    op.tile([128, 1024], f32, tag=f"o{seg}", name=f"o{seg}")
                    for seg in range(4)
                ]
                nh = 2 if halves else 1
                w = 1024 // nh
                for h in range(nh):
                    c0 = w * h
                    for seg, (alu, lo, eng) in enumerate(specs):
                        oh = os_[seg][:, c0 : c0 + w]
                        if eng == "D":
                            nc.vector.tensor_tensor(
                                oh, rc2[:, lo + c0 : lo + c0 + w],
                                rn[:, lo + c0 : lo + c0 + w], alu
                            )
                        else:
                            nc.gpsimd.tensor_tensor(
                                oh, rc2[:, lo + c0 : lo + c0 + w],
                                rn[:, lo + c0 : lo + c0 + w], alu
                            )
                        nc.sync.dma_start(
                            out_d[b0 : b0 + 128, 1024 * seg + c0 : 1024 * seg + c0 + w],
                            oh,
                        )

            def emit_unfold(bt, rCN5, rN10, rTp, rTm, halves=False):
                rn = emit_rn_chain(rTp, rTm)
                rc2 = emit_rc_part(rCN5, rN10)
                emit_finals(bt, rc2, rn, halves=halves)

            def emit_tail_block(f, bt):
                """Last block: T matmuls first (longest reconstruction chain),
                C5/N5 next, nega1024 last in two halves; pre-combines
                preN0 = rc1+rN10h, preN1 = rc1-rN10h feed two finals each, and
                each output half streams out as its rN10 half evacuates."""
                b0 = 128 * bt
                rTp, rTm = mms_t(f)
                rCN5 = mms_cn5(f)
                rn = emit_rn_chain(rTp, rTm)
                rc1 = up.tile([128, 1024], bf16, tag="rc1", name="rc1")
                nc.vector.tensor_tensor(
                    rc1[:, 0:512], rCN5[:, 0:512], rCN5[:, 512:1024], ADD
                )
                nc.vector.tensor_tensor(
                    rc1[:, 512:1024], rCN5[:, 0:512], rCN5[:, 512:1024], SUB
                )
                pN10a = pp.tile([128, 512], f32, tag="pn10a", name="pn10a")
                pN10b = pp.tile([128, 512], f32, tag="pn10b", name="pn10b")
                sN10 = strip["sN10"]
                for j, ps in ((0, pN10a), (1, pN10b)):
                    for a in range(8):
                        v0 = 896 - 128 * a + 512 * j
                        nc.tensor.matmul(
                            ps[:], f["xcn"][:, a, :], sN10[:, v0 : v0 + 512],
                            start=(a == 0), stop=(a == 7),
                        )
                preN = up.tile([128, 2, 1024], bf16, tag="preN", name="preN")
                rN10 = ep.tile([128, 1024], bf16, tag="rn10", name="rn10")
                os_ = [
                    op.tile([128, 1024], f32, tag=f"o{seg}", name=f"o{seg}")
                    for seg in range(4)
                ]
                for h, ps in ((0, pN10a), (1, pN10b)):
                    c0 = 512 * h
                    rh = rN10[:, c0 : c0 + 512]
                    nc.scalar.mul(rh, ps[:], 0.25)
                    nc.vector.tensor_tensor(  # preN0 half
                        preN[:, 0, c0 : c0 + 512], rc1[:, c0 : c0 + 512], rh, ADD
                    )
                    nc.vector.tensor_tensor(  # preN1 half
                        preN[:, 1, c0 : c0 + 512], rc1[:, c0 : c0 + 512], rh, SUB
                    )
                    for seg, eng in ((0, "D"), (1, "P"), (2, "D"), (3, "P")):
                        alu = ADD if seg in (0, 1) else SUB
                        rn_half = rn[:, 1024 * (seg % 2) + c0 : 1024 * (seg % 2) + c0 + 512]
                        oh = os_[seg][:, c0 : c0 + 512]
                        if eng == "D":
                            nc.vector.tensor_tensor(
                                oh, preN[:, seg % 2, c0 : c0 + 512], rn_half, alu
                            )
                        else:
                            nc.gpsimd.tensor_tensor(
                                oh, preN[:, seg % 2, c0 : c0 + 512], rn_half, alu
                            )
                        nc.sync.dma_start(
                            out_d[b0 : b0 + 128, 1024 * seg + c0 : 1024 * seg + c0 + 512],
                            oh,
                        )

            # ---------------- preamble ----------------
            nc.gpsimd.memset(warm[:], 0.0)
            xbig = xbigp.tile([128, 32, 128], f32, tag="xbig", name="xbig")
            nc.sync.dma_start(xbig[:, 0:16, :], xtr_t[:, 0:16, 0:128])
            nc.sync.dma_start(xbig[:, 16:32, :], xtr_t[:, 16:32, 0:128])
            nc.sync.dma_start(SB[:, 0:1792], sb_d[:, 0:1792])           # sC5+sN5
            nc.sync.dma_start(SB[:, 1792:SB_SPLIT], sb_d[:, 1792:SB_SPLIT])  # sN10
            nc.sync.dma_start(SB[:, SB_SPLIT:SBW], sb_d[:, SB_SPLIT:SBW])
            # PE clock warmup: HAM releases 2.4 GHz after ~3us of activity;
            # burn dummies while the first DMAs/folds land.
            pW = pp.tile([128, 512], f32, tag="pc5", name="pwarm")
            for _ in range(25):
                nc.tensor.matmul(
                    pW[:], warm[:, 0:128], warm[:], start=True, stop=True
                )

            # ---------------- main pipeline ----------------
            # Iteration bt interleaves emissions so each engine's in-order
            # stream matches when its work becomes runnable:
            #   mms+evacs(bt) | rn-chain(bt-1) | early folds(bt+1) |
            #   rc-part(bt-1) | T-folds(bt+1) | finals(bt-1)
            f_cur = emit_folds(xbig, first=True)
            xbig_next = emit_dma_in(1)
            r_prev = None
            for bt in range(NB - 1):
                xbig = xbig_next
                if bt + 2 < NB:
                    xbig_next = emit_dma_in(bt + 2)
                rCN5 = mms_cn5(f_cur)
                rN10 = mms_n10(f_cur)
                rTp, rTm = mms_t(f_cur)
                if r_prev is not None:
                    rn = emit_rn_chain(r_prev[2], r_prev[3])
                fa = emit_folds_a(xbig, swap=(bt == 0))
                if r_prev is not None:
                    rc2 = emit_rc_part(r_prev[0], r_prev[1])
                f_cur = emit_folds_b(fa)
                if r_prev is not None:
                    emit_finals(bt - 1, rc2, rn)
                r_prev = (rCN5, rN10, rTp, rTm)
            emit_unfold(NB - 2, *r_prev)
            emit_tail_block(f_cur, NB - 1)

    nc.compile()
    return nc


def _get_nc():
    if "nc" not in _STATE:
        _STATE["nc"] = _build()
    return _STATE["nc"]


def _make_strip_table(w):
    """All 7 leaf band strips, packed [128, SBW] bf16.

    Strip tiles are shears: tile[p, v] = S[OFF + v - p]. Sequences (t any int,
    Ecyc = wt cyclic):
      Ep(t)   = Ecyc(t) + Ecyc(t+2048)          cyc2048
      En(t)   = Ecyc(t) - Ecyc(t+2048)          nega2048
      En10(t) = Ep(t) - Ep(t+1024)              nega1024 leaf
      Epp(t)  = Ep(t) + Ep(t+1024)              cyc1024
      Ec5(t)  = Epp(t) + Epp(t+512)             cyc512 leaf
      En5(t)  = Epp(t) - Epp(t+512)             nega512 leaf
      D(t) = En(t) - En(t+1024); Bt(t) = En(t+512) + En(t+1536)
      L+/-(t) = D(t) +- sqrt2 En(t+1536)        T1024 low strips
      H+/-(t) = Bt(t) -+ sqrt2 En(t+1024)       T1024 high strips
    CRT scales (1/8, 1/8, 1/4, 1/(4sqrt2)) are applied in the ACT evacs.
    """
    import ml_dtypes

    wt = np.roll(w[::-1], 1).astype(np.float64)
    Ecyc = lambda t: wt[np.mod(t, N)]
    Ep = lambda t: Ecyc(t) + Ecyc(t + 2048)
    En = lambda t: Ecyc(t) - Ecyc(t + 2048)
    Epp = lambda t: Ep(t) + Ep(t + 1024)
    seqs = {
        "sC5": lambda t: Epp(t) + Epp(t + 512),
        "sN5": lambda t: Epp(t) - Epp(t + 512),
        "sN10": lambda t: Ep(t) - Ep(t + 1024),
        "sLp": lambda t: (En(t) - En(t + 1024)) + R2 * En(t + 1536),
        "sLm": lambda t: (En(t) - En(t + 1024)) - R2 * En(t + 1536),
        "sHp": lambda t: (En(t + 512) + En(t + 1536)) - R2 * En(t + 1024),
        "sHm": lambda t: (En(t + 512) + En(t + 1536)) + R2 * En(t + 1024),
    }
    p = np.arange(128)[:, None]
    tab = np.zeros((128, SBW), dtype=np.float64)
    for name, (c0, width, off) in STRIPS.items():
        v = np.arange(width)[None, :]
        tab[:, c0 : c0 + width] = seqs[name](off + v - p)
    return np.ascontiguousarray(tab.astype(ml_dtypes.bfloat16))


def _prep_inputs(x, w):
    x = np.ascontiguousarray(x, dtype=np.float32)
    w = np.ascontiguousarray(w, dtype=np.float32)
    sb = _make_strip_table(w)
    # chunk-interleaved layout: position 2t holds ring chunk t, position
    # 2t+1 holds chunk t+16 — L1 fold pairs become stride-2 neighbors, so
    # block 0 can fold each DMA half as it lands.
    perm = [(i >> 1) + 16 * (i & 1) for i in range(32)]
    in_maps = []
    for i in range(N_CORES):
        xtr = np.ascontiguousarray(x[i * B_SHARD : (i + 1) * B_SHARD].T)
        xtr = np.ascontiguousarray(
            xtr.reshape(32, 128, B_SHARD)[perm].reshape(N, B_SHARD)
        )
        in_maps.append({"xtr": xtr, "sb": sb})
    return in_maps


def kernel(x, w, _trace=False):
    from concourse.bass_utils import run_bass_kernel_spmd

    nc = _get_nc()
    in_maps = _prep_inputs(x, w)
    res = run_bass_kernel_spmd(nc, in_maps, list(range(N_CORES)), trace=_trace)
    out = np.concatenate([res.results[i]["out"] for i in range(N_CORES)], axis=0)
    if _trace:
        _STATE["last_result"] = res
    return out
